# revision 1
# baseline (speedup 1.0000x reference)
"""CQAttention Trainium2 kernel: out = concat([C, A, C*A, C*Bv], -1).

Math notes (exact, not approximations):
  - similarity = sub0 + sub1 + sub2 + bias is consumed only through
    softmax over the last axis (m). sub0 (per-row) and bias (scalar) are
    constant along m, so they cancel in both softmaxes -> dropped.
  - With all-ones masks S1 == S2, so Bt = S1 @ S1^T.
  - Softmax normalization is folded into per-partition scales:
      E^T[m,n] = exp(sub1[m] + sub2[n,m])   (no max-subtract; |scores| ~ 12)
      r[n] = 1 / sum_m E[n,m]
      A  = diag(r) (E Q)
      F  = E E^T (symmetric);  F'' = diag(r) F
      Bv = diag(r) (sum_k F''[k,n] C[k,:])
  - All matmuls run as float32r (full-rate fp32 on the trn2 PE). C/Q are
    loaded as f32r-tagged byte copies, so the C passthrough block and the
    C*A / C*Bv multiplies see the exact f32 bits.
  - PE-transpose mode does not count as "busy" for the PE HAM clock gate,
    so transposes are interleaved with real matmuls (dummy warmup matmuls
    in the first phase; batch b+1's transposes inside batch b's (d) loop)
    to keep the array at K=8/8.

Sharding: data-parallel over batch; core i handles batches [2i, 2i+1].
"""

import sys

if "/opt/trn_rl_repo" not in sys.path:
    sys.path.insert(0, "/opt/trn_rl_repo")

import numpy as np

B, N, M, D = 16, 1024, 512, 512
NCORES = 8
BPC = B // NCORES  # batches per core
P = 128
NC = N // P  # 8 n-chunks
MC = M // P  # 4 m-chunks
DC = D // P  # 4 d-chunks

_cache = {}


def _build():
    import concourse.bass as bass
    import concourse.tile as tile
    from concourse import bacc, mybir
    from concourse.masks import make_identity

    f32 = mybir.dt.float32
    f32r = mybir.dt.float32r
    ACT = mybir.ActivationFunctionType

    nc = bacc.Bacc("TRN2")
    Cd = nc.dram_tensor("C", [BPC, N, D], f32, kind="ExternalInput")
    Qd = nc.dram_tensor("Q", [BPC, M, D], f32, kind="ExternalInput")
    w4Qd = nc.dram_tensor("w4Q", [D, 1], f32, kind="ExternalInput")
    wmlud = nc.dram_tensor("wmlu", [1, 1, D], f32, kind="ExternalInput")
    outd = nc.dram_tensor("out", [BPC, N, 4 * D], f32, kind="ExternalOutput")
    rs_dram = nc.dram_tensor("rs_scratch", [BPC, N], f32, kind="Internal")

    with tile.TileContext(nc) as tc:
        with (
            tc.tile_pool(name="consts", bufs=1) as consts,
            tc.tile_pool(name="cq", bufs=2) as cq,
            tc.tile_pool(name="work", bufs=1) as work,
            tc.tile_pool(name="stage", bufs=3) as stage,
            tc.tile_pool(name="ps_f", bufs=2, space="PSUM") as ps_f,
            tc.tile_pool(name="ps_acc", bufs=2, space="PSUM") as ps_acc,
            tc.tile_pool(name="ps_mid", bufs=2, space="PSUM") as ps_mid,
        ):
            ident = consts.tile([P, P], f32, tag="ident")
            make_identity(nc, ident)
            ident_r = consts.tile([P, P], f32r, tag="identr")
            nc.vector.tensor_copy(out=ident_r, in_=ident)
            w4Qb = consts.tile([P, D], f32, tag="w4Qb")
            nc.gpsimd.dma_start(
                out=w4Qb, in_=bass.AP(tensor=w4Qd, offset=0, ap=[[0, P], [1, D]])
            )
            wmlu_pp = consts.tile([P, DC], f32, tag="wmlu")
            nc.gpsimd.dma_start(
                out=wmlu_pp, in_=bass.AP(tensor=wmlud, offset=0, ap=[[1, P], [P, DC]])
            )
            ones_f32 = consts.tile([P, 8], f32, tag="ones32")
            nc.vector.memset(ones_f32, 1.0)
            ones_col = consts.tile([P, 8], f32r, tag="ones")
            nc.vector.tensor_copy(out=ones_col, in_=ones_f32)

            def alloc_and_load(b):
                """DMA C/Q for batch b as f32r-tagged byte copies + the C
                passthrough output block."""
                tl = {"b": b}
                tl["C_r"] = cq.tile([P, NC, D], f32r, tag="cr", name="C_r")
                tl["Q_r"] = cq.tile([P, MC, D], f32r, tag="qr", name="Q_r")
                for c in range(NC):
                    nc.sync.dma_start(
                        out=tl["C_r"][:, c, :],
                        in_=Cd[b, c * P : (c + 1) * P, :].bitcast(f32r),
                    )
                for mm in range(MC):
                    nc.sync.dma_start(
                        out=tl["Q_r"][:, mm, :],
                        in_=Qd[b, mm * P : (mm + 1) * P, :].bitcast(f32r),
                    )
                nc.sync.dma_start(
                    out=outd[b, :, 0:512].rearrange("(c p) d -> p c d", p=P),
                    in_=tl["C_r"].bitcast(f32),
                )
                tl["CT"] = work.tile([P, DC, N], f32r, tag="ct", name="CT")
                tl["QwT"] = work.tile([P, DC, M], f32r, tag="qwt", name="QwT")
                tl["ET"] = work.tile([P, MC, N], f32r, tag="et", name="ET")
                tl["F"] = work.tile([P, NC, N], f32r, tag="f", name="F")
                tl["sub1"] = work.tile([P, MC], f32, tag="sub1", name="sub1")
                tl["rr"] = work.tile([P, NC], f32, tag="r", name="rr")
                tl["scr"] = work.tile([P, D], f32, tag="scr", name="scr")
                tl["rs_row"] = work.tile([1, N], f32, tag="rsrow", name="rs_row")
                return tl

            def gen_transposes(tl, pools):
                """Yield after each PE transpose + psum-drain copy."""
                C_r, Q_r, CT, QwT = tl["C_r"], tl["Q_r"], tl["CT"], tl["QwT"]
                i = 0
                for c in range(NC):
                    for e in range(DC):
                        pool, tag = pools[i % len(pools)]
                        tp = pool.tile([P, P], f32r, tag=tag)
                        nc.tensor.transpose(
                            tp, C_r[:, c, e * P : (e + 1) * P], ident_r
                        )
                        nc.vector.tensor_copy(
                            out=CT[:, e, c * P : (c + 1) * P], in_=tp
                        )
                        i += 1
                        yield
                for mm in range(MC):
                    for e in range(DC):
                        pool, tag = pools[i % len(pools)]
                        tp = pool.tile([P, P], f32r, tag=tag)
                        nc.tensor.transpose(
                            tp, Q_r[:, mm, e * P : (e + 1) * P], ident_r
                        )
                        # scale rows (=d) by w4mlu[d] while draining psum
                        nc.vector.tensor_scalar_mul(
                            out=QwT[:, e, mm * P : (mm + 1) * P],
                            in0=tp,
                            scalar1=wmlu_pp[:, e : e + 1],
                        )
                        i += 1
                        yield

            def emit_sub1(tl):
                # sub1[m] = sum_d Q[m,d] * w4Q[d], per-partition layout
                for mm in range(MC):
                    nc.vector.tensor_mul(
                        out=tl["scr"],
                        in0=tl["Q_r"][:, mm, :].bitcast(f32),
                        in1=w4Qb,
                    )
                    nc.vector.reduce_sum(
                        out=tl["sub1"][:, mm : mm + 1],
                        in_=tl["scr"],
                        axis=mybir.AxisListType.X,
                    )

            def emit_a(tl):
                # sim^T -> E^T = exp(sim^T + sub1); rowsum per n-half
                b = tl["b"]
                CT, QwT, ET = tl["CT"], tl["QwT"], tl["ET"]
                for nh in range(2):
                    for mm in range(MC):
                        sim_ps = ps_mid.tile([P, 512], f32, tag="mid")
                        for e in range(DC):
                            nc.tensor.matmul(
                                sim_ps,
                                lhsT=QwT[:, e, mm * P : (mm + 1) * P],
                                rhs=CT[:, e, nh * 512 : (nh + 1) * 512],
                                start=(e == 0),
                                stop=(e == DC - 1),
                            )
                        nc.scalar.activation(
                            out=ET[:, mm, nh * 512 : (nh + 1) * 512],
                            in_=sim_ps,
                            func=ACT.Exp,
                            bias=tl["sub1"][:, mm : mm + 1],
                            scale=1.0,
                        )
                    rsT_ps = ps_acc.tile([8, 512], f32, tag="acc")
                    for e in range(MC):
                        nc.tensor.matmul(
                            rsT_ps,
                            lhsT=ones_col,
                            rhs=ET[:, e, nh * 512 : (nh + 1) * 512],
                            start=(e == 0),
                            stop=(e == MC - 1),
                        )
                    nc.vector.tensor_copy(
                        out=tl["rs_row"][:, nh * 512 : (nh + 1) * 512],
                        in_=rsT_ps[0:1, :],
                    )
                # re-layout [1, N] -> per-partition [P, NC] via DRAM bounce
                nc.sync.dma_start(out=rs_dram[b], in_=tl["rs_row"][0:1, :])
                rs_pp = stage.tile([P, NC], f32, tag="rspp")
                nc.sync.dma_start(
                    out=rs_pp, in_=rs_dram[b].rearrange("(c p) -> p c", p=P)
                )
                nc.vector.reciprocal(out=tl["rr"], in_=rs_pp)

            def emit_bc(tl):
                # F row-blocks (scaled by r) and A = diag(r) E Q
                b = tl["b"]
                ET, Q_r, C_r = tl["ET"], tl["Q_r"], tl["C_r"]
                F, rr = tl["F"], tl["rr"]
                for c in range(NC):
                    F_ps = ps_f.tile([P, N], f32, tag="f")
                    A_ps = ps_acc.tile([P, 512], f32, tag="acc")
                    for e in range(MC):
                        lhs = ET[:, e, c * P : (c + 1) * P]
                        st, sp = (e == 0), (e == MC - 1)
                        nc.tensor.matmul(
                            F_ps[:, 0:512],
                            lhsT=lhs,
                            rhs=ET[:, e, 0:512],
                            start=st,
                            stop=sp,
                        )
                        nc.tensor.matmul(
                            F_ps[:, 512:1024],
                            lhsT=lhs,
                            rhs=ET[:, e, 512:1024],
                            start=st,
                            stop=sp,
                        )
                        nc.tensor.matmul(
                            A_ps, lhsT=lhs, rhs=Q_r[:, e, :], start=st, stop=sp
                        )
                    # F'' = diag(r) F on ACT, split in halves for latency
                    nc.scalar.activation(
                        out=F[:, c, 0:512],
                        in_=F_ps[:, 0:512],
                        func=ACT.Copy,
                        scale=rr[:, c : c + 1],
                    )
                    nc.scalar.activation(
                        out=F[:, c, 512:1024],
                        in_=F_ps[:, 512:1024],
                        func=ACT.Copy,
                        scale=rr[:, c : c + 1],
                    )
                    # A = diag(r) (E Q); C*A
                    A_s = stage.tile([P, 512], f32, tag="a")
                    nc.scalar.activation(
                        out=A_s, in_=A_ps, func=ACT.Copy, scale=rr[:, c : c + 1]
                    )
                    CA_s = stage.tile([P, 512], f32, tag="ca")
                    nc.vector.tensor_mul(
                        out=CA_s, in0=C_r[:, c, :].bitcast(f32), in1=A_s
                    )
                    nc.sync.dma_start(
                        out=outd[b, c * P : (c + 1) * P, 512:1024], in_=A_s
                    )
                    nc.sync.dma_start(
                        out=outd[b, c * P : (c + 1) * P, 1024:1536], in_=CA_s
                    )

            def emit_d(tl, interleave=None):
                # Bv = diag(r) sum_k F''[k, n] C[k, :]
                b = tl["b"]
                C_r, F, rr = tl["C_r"], tl["F"], tl["rr"]
                for c in range(NC):
                    Bv_ps = ps_acc.tile([P, 512], f32, tag="acc")
                    for i in range(NC):
                        nc.tensor.matmul(
                            Bv_ps,
                            lhsT=F[:, i, c * P : (c + 1) * P],
                            rhs=C_r[:, i, :],
                            start=(i == 0),
                            stop=(i == NC - 1),
                        )
                    Bv_s = stage.tile([P, 512], f32, tag="bv")
                    nc.vector.tensor_scalar_mul(
                        out=Bv_s, in0=Bv_ps, scalar1=rr[:, c : c + 1]
                    )
                    CBv_s = stage.tile([P, 512], f32, tag="cbv")
                    nc.vector.tensor_mul(
                        out=CBv_s, in0=C_r[:, c, :].bitcast(f32), in1=Bv_s
                    )
                    nc.sync.dma_start(
                        out=outd[b, c * P : (c + 1) * P, 1536:2048], in_=CBv_s
                    )
                    if interleave is not None:
                        for _ in range(6):
                            next(interleave, None)

            def emit_warm_mm(junk_ps):
                # real (non-transpose) matmul to keep the PE HAM clock warm
                nc.tensor.matmul(
                    junk_ps, lhsT=ident_r, rhs=ident_r, start=True, stop=True
                )

            # ---- pipeline over the two batches ----
            tl0 = alloc_and_load(0)
            junk_ps = ps_f.tile([P, P], f32, tag="f")
            gen0 = gen_transposes(tl0, [(ps_mid, "mid"), (ps_acc, "acc")])
            for i, _ in enumerate(gen0):
                if i % 3 == 2:
                    emit_warm_mm(junk_ps)
            emit_sub1(tl0)
            emit_a(tl0)
            emit_bc(tl0)

            tl1 = alloc_and_load(1)
            gen1 = gen_transposes(tl1, [(ps_mid, "mid")])
            emit_d(tl0, interleave=gen1)
            for _ in gen1:
                pass
            emit_sub1(tl1)
            emit_a(tl1)
            emit_bc(tl1)
            emit_d(tl1)

    nc.compile()
    return nc


def _reference_fallback(C, Q, Cmask, Qmask, w4C, w4Q, w4mlu, bias):
    """Numpy fallback for non-all-ones masks (not expected per spec)."""

    def softmax(x, axis):
        x = x - np.max(x, axis=axis, keepdims=True)
        e = np.exp(x)
        return e / np.sum(e, axis=axis, keepdims=True)

    sub0 = C @ w4C
    sub1 = np.swapaxes(Q @ w4Q, 1, 2)
    sub2 = np.einsum("bnd,bmd->bnm", C * w4mlu, Q)
    sim = sub0 + sub1 + sub2 + bias
    s1m = np.where(Qmask[:, None, :] == 0, -np.inf, sim)
    s2m = np.where(Cmask[:, :, None] == 0, -np.inf, sim)
    S1 = softmax(s1m, -1)
    S2 = softmax(s2m, -1)
    A = np.einsum("bnm,bmd->bnd", S1, Q)
    Bt = np.einsum("bnm,bkm->bnk", S1, S2)
    Bv = np.einsum("bnk,bkd->bnd", Bt, C)
    return np.concatenate([C, A, C * A, C * Bv], axis=2).astype(np.float32)


def kernel(C, Q, Cmask, Qmask, w4C, w4Q, w4mlu, bias):
    C = np.asarray(C, np.float32)
    Q = np.asarray(Q, np.float32)
    w4Q = np.asarray(w4Q, np.float32)
    w4mlu = np.asarray(w4mlu, np.float32)

    if not (np.all(np.asarray(Cmask) == 1) and np.all(np.asarray(Qmask) == 1)):
        return _reference_fallback(
            C,
            Q,
            np.asarray(Cmask),
            np.asarray(Qmask),
            np.asarray(w4C, np.float32),
            w4Q,
            w4mlu,
            np.asarray(bias, np.float32),
        )

    import os

    from concourse.bass_utils import run_bass_kernel_spmd

    if "nc" not in _cache:
        _cache["nc"] = _build()
    nc = _cache["nc"]

    in_maps = []
    for i in range(NCORES):
        in_maps.append(
            {
                "C": np.ascontiguousarray(C[i * BPC : (i + 1) * BPC]),
                "Q": np.ascontiguousarray(Q[i * BPC : (i + 1) * BPC]),
                "w4Q": np.ascontiguousarray(w4Q),
                "wmlu": np.ascontiguousarray(w4mlu),
            }
        )

    trace = bool(int(os.environ.get("BASS_KERNEL_TRACE", "0")))
    res = run_bass_kernel_spmd(
        nc, in_maps, core_ids=list(range(NCORES)), trace=trace
    )
    if trace:
        _cache["exec_time_ns"] = res.exec_time_ns
        _cache["trace"] = res.instructions_and_trace
    out = np.concatenate([r["out"] for r in res.results], axis=0)
    return out

